# revision 46
# baseline (speedup 1.0000x reference)
"""Trainium2 Bass kernel for nn_Derivative_78898549227959 (gnn_message_passing).

Computes, for x = where(discrete_mask, (inputs > 0), inputs)  [straight-through
forward value], per-node tiny MLPs with adjacency-masked inputs:

    h1 = relu(einsum('bd,ndh->bnh', x, A[n,d]*W1[n,d,h]) + b1)
    h2 = relu(einsum('bnh,nhk->bnk', h1, W2) + b2)
    out[b,n] = einsum('bnk,nk->bn', h2, W3) + b3

Distribution: data-parallel over 8 NeuronCores - batch B=8192 sharded into
8 x 1024; weights/adjacency replicated (SPMD, same program each core).

Host-side prep is pure layout (transpose/pad/gather/permutation + fp16
rounding). All arithmetic - adjacency masking of W1, input binarization,
matmuls, biases, relus - runs on device.

v2 kernel structure (per core, BS=1024), changes vs the 151us baseline:

 - L1 contracts K=128 (rows d=0..127) in ONE matmul pass per 512-chunk
   instead of two 65-row passes: halves L1 PE streaming. The remaining
   d=128 row is handled exactly by:
     * a host-side PERMUTATION of each core's batch into
       [128 'mixed' | 384 'c0-type' | 512 'c1-type'] by the binary value
       of x[:,128] (d=128 is a discrete column, so its straight-through
       value is exactly 0/1). c1 is the globally-majority bit, c0 per-core.
     * a K=1 'mini matmul' (N=128) accumulating (x128-c0)*w1last into the
       mixed region's PSUM, where w1last = A[n,128]*W1[128,n,:] is masked
       on device and staged per-pair via tiny SBUF->SBUF DMAs.
     * per-partition eviction biases bias0 = b1 + c0*w1last and
       bias1 = b1 + c1*w1last (computed on device from 0/1 selector
       inputs), free on the ACT relu eviction.
   This is exact for arbitrary inputs.
 - L2 runs as four concurrent 64x64 matmuls via tile_position quadrants
   (two node pairs per pass): halves L2 PE streaming. W2 is packed dense.
   The odd pair's h2 lands partition-swapped (node 2j+1 in partitions
   0:64); b2/W3 packing bakes in the swap.
 - L3 unchanged: sparse-W3 [128,128] lhsT per pair accumulating into a
   pinned [128,1024] PSUM giving outT[n,b] directly.
 - The quad loop is software-pipelined: iteration i emits L1(quad i),
   L2(quad i-1), L3(quad i-2), so PSUM fits in 8 banks (L1: 2x[128,512],
   L2: 2x[128,1024], L3: [128,1024]) without pipeline stalls.
 - PE instruction order is pinned with ordering-only dep edges (the tile
   scheduler otherwise breaks weight reuse / quadrant grouping); all DMA
   triggers ride the in-order SP ring, priority-ordered (w1last chain,
   x, chunk-0 W1, small tables, bulk weights), since each trigger costs
   ~600ns of issuing-engine queue time and the ACT/DVE queues are needed
   for evictions.

Measured on 8 axon-tunneled TRN2 cores: ~138-146 us HW exec (chip
power-state adds ~±5% run-to-run), vs 150 us for the previous baseline;
relative error ~5.3e-4 against the fp32 CPU reference (tolerance 2e-2).
"""

import sys

sys.path.insert(0, "/opt/trn_rl_repo")

import numpy as np

import concourse.bacc as bacc
import concourse.mybir as mybir
from concourse.bass_utils import run_bass_kernel_spmd
from concourse.tile import TileContext, add_dep_helper

B = 8192
D = 129
H = 64
N_CORES = 8
BS = B // N_CORES          # 1024 batch rows per core
NPAIR = 64                 # node pairs (0..127); node 128 is pair index 64
NQUAD = 32
MIXED = 128                # mixed region columns [0:128)
C0N = 384                  # c0-type region [128:512)
C1N = 512                  # c1-type region [512:1024)
F32 = mybir.dt.float32
F16 = mybir.dt.float16

AF = mybir.ActivationFunctionType
OP = mybir.AluOpType


def build():
    nc = bacc.Bacc("TRN2", target_bir_lowering=False, debug=False,
                   num_devices=N_CORES)

    d_xt = nc.dram_tensor("xt_raw", [128, BS], F16, kind="ExternalInput")
    d_x128m = nc.dram_tensor("x128m_raw", [1, MIXED], F16,
                             kind="ExternalInput")
    d_mt = nc.dram_tensor("mt", [128, 1], F32, kind="ExternalInput")
    d_c0 = nc.dram_tensor("c0rep", [128, 1], F32, kind="ExternalInput")
    d_c1 = nc.dram_tensor("c1rep", [128, 1], F32, kind="ExternalInput")
    d_w1 = nc.dram_tensor("W1pk", [128, 130 * H], F16, kind="ExternalInput")
    d_atpk = nc.dram_tensor("atpk", [128, 130], F16, kind="ExternalInput")
    d_w1lT = nc.dram_tensor("w1lT_raw", [65, 128], F16, kind="ExternalInput")
    d_at128T = nc.dram_tensor("at128T", [65, 128], F16, kind="ExternalInput")
    d_w1l = nc.dram_tensor("w1l_raw", [128, 65], F32, kind="ExternalInput")
    d_at128 = nc.dram_tensor("at128", [128, 65], F32, kind="ExternalInput")
    d_b1pk = nc.dram_tensor("b1pack", [128, 65], F32, kind="ExternalInput")
    d_w2 = nc.dram_tensor("W2pk", [128, 65 * H], F16, kind="ExternalInput")
    d_b2pk = nc.dram_tensor("b2pack", [128, 65], F32, kind="ExternalInput")
    d_w3 = nc.dram_tensor("W3pk", [128, 65 * 128], F16, kind="ExternalInput")
    d_b3col = nc.dram_tensor("b3col", [128, 1], F32, kind="ExternalInput")
    d_b3hi = nc.dram_tensor("b3hi", [1, 1], F32, kind="ExternalInput")
    d_outT = nc.dram_tensor("outT", [D, BS], F32, kind="ExternalOutput")

    with TileContext(nc) as tc:
        with tc.tile_pool(name="consts", bufs=1) as consts:
            xt_raw = consts.tile([128, BS], F16)
            x128m_raw = consts.tile([1, MIXED], F16)
            mt = consts.tile([128, 1], F32)
            c0rep = consts.tile([128, 1], F32)
            c1rep = consts.tile([128, 1], F32)
            w1 = consts.tile([128, 130 * H], F16)
            atpk = consts.tile([128, 130], F16)
            w1lT_raw = consts.tile([65, 128], F16)
            at128T = consts.tile([65, 128], F16)
            w1l_raw = consts.tile([128, 65], F32)
            at128 = consts.tile([128, 65], F32)
            b1pk = consts.tile([128, 65], F32)
            w2 = consts.tile([128, 65 * H], F16)
            b2pk = consts.tile([128, 65], F32)
            w3 = consts.tile([128, 65 * 128], F16)
            b3col = consts.tile([128, 1], F32)
            b3hi = consts.tile([1, 1], F32)

            sgn = consts.tile([128, BS], F16)
            xt = consts.tile([128, BS], F16)
            x128adj = consts.tile([1, MIXED], F16)
            w1lT = consts.tile([65, 128], F16)     # masked, for mini-MM lhsT
            w1l = consts.tile([128, 65], F32)      # masked, for biases
            bias0 = consts.tile([128, 65], F32)    # b1 + c0*w1l
            bias1 = consts.tile([128, 65], F32)    # b1 + c1*w1l
            outT = consts.tile([128, BS], F32)
            outThi = consts.tile([1, BS], F32)
            zcol = consts.tile([128, 1], F32)

            # ---------------- DMA staging (stagger for startup) ----------
            # Critical path to the first matmul: xt_raw + mt (sync ring),
            # then ACT signs, then the DVE straight-through chain and the
            # chunk-0 W1 masking. Keep both trigger queues short before
            # the compute that gates pair 0; bulk weights stream after.
            w1q = [slice(q * 16 * H, min((q + 1) * 16, 130) * H)
                   for q in range(9)]                  # 16-halfblock chunks
            w3q = [slice(0, 16 * 128), slice(16 * 128, 40 * 128),
                   slice(40 * 128, 65 * 128)]
            w2q = [slice(0, 24 * H), slice(24 * H, 65 * H)]

            # DMA trigger cost is ~600ns of issuing-engine queue time and
            # the SP ring's hw queue completes transfers IN ORDER, so the
            # ring doubles as a priority list: w1lT (gates the stageAll
            # flatten) and xt (gates everything) first, bulk weights last.
            # No triggers on the ACT ring - it carries signs + evictions.
            nc.sync.dma_start(out=w1lT_raw, in_=d_w1lT.ap())
            nc.sync.dma_start(out=at128T, in_=d_at128T.ap())
            nc.sync.dma_start(out=x128m_raw, in_=d_x128m.ap())
            nc.sync.dma_start(out=xt_raw, in_=d_xt.ap())
            nc.sync.dma_start(out=mt, in_=d_mt.ap())
            nc.sync.dma_start(out=atpk, in_=d_atpk.ap())
            nc.sync.dma_start(out=w1[:, w1q[0]], in_=d_w1.ap()[:, w1q[0]])
            nc.sync.dma_start(out=c0rep, in_=d_c0.ap())
            nc.sync.dma_start(out=c1rep, in_=d_c1.ap())

            # mask w1last early and flatten all 65 rows onto partition 0
            # with ONE SBUF->SBUF DMA; the K=1 mini matmuls slice it
            stageAll = consts.tile([1, 65 * 128], F16)
            nc.vector.tensor_tensor(w1lT, w1lT_raw, at128T, OP.mult)
            nc.sync.dma_start(out=stageAll, in_=w1lT)

            # ---------------- W1 chunk-0 masking + x preprocessing -------
            # DVE queue order = critical path to pair 0's first matmul:
            # chunk-0 mask (independent of x), then hard = max(sign(x), 0);
            # x = (hard - x)*m + x, split into batch halves (the first
            # matmul only needs xt[:, 0:512]).
            w1r = w1.rearrange("p (m h) -> p m h", m=130)

            def mask_chunk(ci, eng):
                m0 = ci * 16
                cnt = min(16, 130 - m0)
                return eng.tensor_tensor(
                    w1r[:, m0:m0 + cnt, :], w1r[:, m0:m0 + cnt, :],
                    atpk[:, m0:m0 + cnt, None].broadcast_to([128, cnt, H]),
                    OP.mult)

            # first mask op covers only the halfblocks pairs 0-1 need;
            # the rest of chunk 0 follows right behind
            w1r0 = w1r[:, 0:4, :]
            nc.vector.tensor_tensor(
                w1r0, w1r0, atpk[:, 0:4, None].broadcast_to([128, 4, H]),
                OP.mult)
            s128 = consts.tile([1, MIXED], F16)
            h0, h1s = slice(0, 512), slice(512, BS)
            nc.scalar.sign(sgn[:, h0], xt_raw[:, h0])
            nc.scalar.sign(s128, x128m_raw)
            nc.vector.scalar_tensor_tensor(sgn[:, h0], sgn[:, h0], 0.0,
                                           xt_raw[:, h0], OP.max, OP.subtract)
            nc.vector.scalar_tensor_tensor(
                xt[:, h0], sgn[:, h0], mt, xt_raw[:, h0], OP.mult, OP.add)
            w1r0b = w1r[:, 4:16, :]
            nc.vector.tensor_tensor(
                w1r0b, w1r0b, atpk[:, 4:16, None].broadcast_to([128, 12, H]),
                OP.mult)
            nc.scalar.sign(sgn[:, h1s], xt_raw[:, h1s])
            nc.vector.memset(zcol, 0.0)
            nc.vector.scalar_tensor_tensor(
                sgn[:, h1s], sgn[:, h1s], 0.0, xt_raw[:, h1s],
                OP.max, OP.subtract)
            nc.vector.scalar_tensor_tensor(
                xt[:, h1s], sgn[:, h1s], mt, xt_raw[:, h1s], OP.mult, OP.add)

            # remaining small input DMAs on SP, then bulk weights (their
            # triggers queue behind stageAll's short mask-gated wait)
            nc.sync.dma_start(out=w1l_raw, in_=d_w1l.ap())
            nc.sync.dma_start(out=at128, in_=d_at128.ap())
            nc.sync.dma_start(out=b1pk, in_=d_b1pk.ap())
            nc.sync.dma_start(out=b2pk, in_=d_b2pk.ap())
            nc.sync.dma_start(out=b3col, in_=d_b3col.ap())
            nc.sync.dma_start(out=b3hi, in_=d_b3hi.ap())
            nc.sync.dma_start(out=w2[:, w2q[0]], in_=d_w2.ap()[:, w2q[0]])
            nc.sync.dma_start(out=w3[:, w3q[0]], in_=d_w3.ap()[:, w3q[0]])
            w1rest = slice(16 * H, 130 * H)
            nc.sync.dma_start(out=w1[:, w1rest], in_=d_w1.ap()[:, w1rest])
            nc.sync.dma_start(out=w2[:, w2q[1]], in_=d_w2.ap()[:, w2q[1]])
            nc.sync.dma_start(out=w3[:, w3q[1]], in_=d_w3.ap()[:, w3q[1]])
            nc.sync.dma_start(out=w3[:, w3q[2]], in_=d_w3.ap()[:, w3q[2]])

            # ---------------- small weight prep --------------------------
            # x128adj = max(sign(x128), 0) - c0   (values in {-c0, 1-c0});
            # gates only the pair-0 mini matmul / evictions, so emitted
            # after the first-matmul critical path. w1l mask on GpSimd
            # (stt variants are DVE-only; Pool has no TensorScalarPtr and
            # no PSUM access); chunks 1-8 of the W1 masking run on DVE
            # (in-loop) / GpSimd in the background.
            nc.vector.scalar_tensor_tensor(
                x128adj, s128, 0.0,
                c0rep[0:1, 0:1].broadcast_to([1, MIXED]), OP.max, OP.subtract)
            nc.gpsimd.tensor_tensor(w1l, w1l_raw, at128, OP.mult)
            nc.vector.scalar_tensor_tensor(bias0, w1l, c0rep, b1pk,
                                           OP.mult, OP.add)
            nc.vector.scalar_tensor_tensor(bias1, w1l, c1rep, b1pk,
                                           OP.mult, OP.add)
            for ci in range(2, 9):
                mask_chunk(ci, nc.gpsimd)

            # ---------------- main software-pipelined loop ---------------
            with (
                tc.tile_pool(name="ps1", bufs=2, space="PSUM") as ps1,
                tc.tile_pool(name="ps2", bufs=1, space="PSUM") as ps2,
                tc.tile_pool(name="ps3", bufs=1, space="PSUM") as ps3,
                tc.tile_pool(name="work", bufs=3) as work,
            ):
                psum3 = ps3.tile([128, BS], F32, name="psum3")
                h1t = {}   # pair j -> h1 tile
                h2t = {}   # pair j -> h2 tile
                pe_last = [None]

                def mm(*args, **kwargs):
                    # matmul with an ordering-only chain so the tile
                    # scheduler preserves the emitted PE instruction order
                    # (keeps weight reuse + quadrant groups intact)
                    i = nc.tensor.matmul(*args, **kwargs)
                    if pe_last[0] is not None:
                        add_dep_helper(i.ins, pe_last[0].ins, sync=False,
                                       reason="pe order")
                    pe_last[0] = i
                    return i

                def emit_l1(j):
                    # pair j: nodes (2j, 2j+1); j == 64: node 128 (M=64)
                    m = 128 if j < NPAIR else 64
                    lhs = w1[:, j * 128:j * 128 + m]
                    h1 = work.tile([128, BS], F16, tag=f"h1{j % 2}",
                                   name=f"h1_{j}")
                    p0 = ps1.tile([128, 512], F32, tag="psum1",
                                  name=f"p1a_{j}")
                    p1 = ps1.tile([128, 512], F32, tag="psum1",
                                  name=f"p1b_{j}")
                    # both main matmuls share lhs (one weight load), the
                    # K=1 mini matmul accumulates the mixed-region d=128
                    # term afterwards
                    mm(p0[0:m], lhs, xt[:, 0:512], start=True, stop=False)
                    mm(p1[0:m], lhs, xt[:, 512:1024], start=True, stop=True)
                    mm(p0[0:m, 0:MIXED],
                       stageAll[0:1, j * 128:j * 128 + m], x128adj,
                       start=False, stop=True)
                    # evictions: relu with per-partition bias (exact)
                    nc.scalar.activation(h1[0:m, 0:512], p0[0:m], AF.Relu,
                                         bias=bias0[0:m, j:j + 1])
                    nc.vector.tensor_scalar(h1[0:m, 512:1024], p1[0:m],
                                            bias1[0:m, j:j + 1], 0.0,
                                            OP.add, OP.max)
                    h1t[j] = h1

                def emit_l2_quad(q):
                    ja, jb = 2 * q, 2 * q + 1
                    h1a, h1b = h1t.pop(ja), h1t.pop(jb)
                    pA = ps2.tile([128, BS], F32, tag="psA", name=f"pA_{q}")
                    pB = ps2.tile([128, BS], F32, tag="psB", name=f"pB_{q}")
                    wa, wb = ja * H, jb * H
                    for bc in range(2):
                        s = slice(bc * 512, (bc + 1) * 512)
                        mm(pA[0:64, s], w2[0:64, wa:wa + H], h1a[0:64, s])
                        mm(pA[64:128, s], w2[64:128, wa:wa + H],
                           h1a[64:128, s])
                        mm(pB[64:128, s], w2[0:64, wb:wb + H], h1b[0:64, s])
                        mm(pB[0:64, s], w2[64:128, wb:wb + H],
                           h1b[64:128, s])
                    h2a = work.tile([128, BS], F16, tag="h2a", name=f"h2a{q}")
                    h2b = work.tile([128, BS], F16, tag="h2b", name=f"h2b{q}")
                    # h2a eviction on ACT, h2b on DVE (GpSimd cannot
                    # access PSUM on TRN2)
                    nc.scalar.activation(h2a, pA, AF.Relu,
                                         bias=b2pk[:, ja:ja + 1])
                    nc.vector.tensor_scalar(h2b, pB, b2pk[:, jb:jb + 1],
                                            0.0, OP.add, OP.max)
                    h2t[ja], h2t[jb] = h2a, h2b

                def emit_l2_tail():
                    h1n = h1t.pop(NPAIR)
                    pA = ps2.tile([128, BS], F32, tag="psA", name="pA_t")
                    wn = NPAIR * H
                    for bc in range(2):
                        s = slice(bc * 512, (bc + 1) * 512)
                        mm(pA[0:64, s], w2[0:64, wn:wn + H], h1n[0:64, s])
                    h2n = work.tile([128, BS], F16, tag="h2a", name="h2n")
                    nc.scalar.activation(h2n[0:64], pA[0:64], AF.Relu,
                                         bias=b2pk[0:64, NPAIR:NPAIR + 1])
                    h2t[NPAIR] = h2n

                def emit_l3(j):
                    # matmul out cannot cross a PSUM bank: two N=512 passes
                    h2 = h2t.pop(j)
                    for bc in range(2):
                        s = slice(bc * 512, (bc + 1) * 512)
                        mm(psum3[:, s], w3[:, j * 128:(j + 1) * 128],
                           h2[:, s], start=(j == 0),
                           stop=(j == 2 * NQUAD - 1))

                def emit_l3_tail():
                    # [1, 512] tail accumulators live in a recycled ps2
                    # tile - runs concurrently with the last L3 pairs
                    h2n = h2t.pop(NPAIR)
                    ph = ps2.tile([128, BS], F32, tag="psA", name="p3h")
                    for bc in range(2):
                        s = slice(bc * 512, (bc + 1) * 512)
                        mm(ph[0:1, s],
                           w3[0:64, NPAIR * 128:NPAIR * 128 + 1],
                           h2n[0:64, s], start=True, stop=True,
                           skip_group_check=True)
                        nc.vector.tensor_scalar_add(outThi[:, s],
                                                    ph[0:1, s], b3hi)

                # pipeline: iter i emits L1(quad i) interleaved around
                # L2(quad i-1), then L3(quad i-2) - keeps every PSUM pool
                # at its minimum buffer count without PE stalls
                for i in range(NQUAD + 2):
                    if i < NQUAD:
                        emit_l1(2 * i)
                    elif i == NQUAD:
                        emit_l1(NPAIR)       # tail node 128
                    if 1 <= i <= NQUAD:
                        emit_l2_quad(i - 1)
                    elif i == NQUAD + 1:
                        emit_l2_tail()
                    if i < NQUAD:
                        emit_l1(2 * i + 1)
                    if 2 <= i <= NQUAD + 1:
                        emit_l3(2 * (i - 2))
                        emit_l3(2 * (i - 2) + 1)
                    if i == 1:
                        mask_chunk(1, nc.vector)

                emit_l3_tail()

                # evict psum3 (+b3) on both engines concurrently, overlap
                # with the output DMAs
                nc.scalar.activation(outT[:, 0:512], psum3[:, 0:512],
                                     AF.Identity, bias=b3col)
                nc.sync.dma_start(out=d_outT.ap()[0:128, 0:512],
                                  in_=outT[:, 0:512])
                nc.vector.tensor_scalar_add(outT[:, 512:1024],
                                            psum3[:, 512:1024], b3col)
                nc.sync.dma_start(out=d_outT.ap()[0:128, 512:1024],
                                  in_=outT[:, 512:1024])

            nc.sync.dma_start(out=d_outT.ap()[128:129], in_=outThi)

            nc._dbg = dict(xt=xt, w1=w1, w2=w2, w3=w3, w1lT=w1lT,
                           bias0=bias0, bias1=bias1, outT=outT,
                           outThi=outThi)

    nc.compile()
    return nc


_NC_CACHE = None


def get_nc():
    global _NC_CACHE
    if _NC_CACHE is None:
        _NC_CACHE = build()
    return _NC_CACHE


def _plan_shards(x128bit):
    """Choose per-core row permutations [128 mixed | 384 c0 | 512 c1].

    c1 is the global majority bit value; c0 is per-core (minority while the
    minority pool lasts, else majority). Returns (perms, c0s, c1s)."""
    idx1 = np.flatnonzero(x128bit)
    idx0 = np.flatnonzero(~x128bit)
    maj = 1 if len(idx1) >= len(idx0) else 0
    pool_maj = list(idx1 if maj == 1 else idx0)
    pool_min = list(idx0 if maj == 1 else idx1)
    c1_blocks, c0_blocks, c0s, mixed_blocks = [], [], [], []
    for c in range(N_CORES):
        c1_blocks.append(pool_maj[:C1N])
        del pool_maj[:C1N]
    for c in range(N_CORES):
        if len(pool_min) >= C0N:
            c0_blocks.append(pool_min[:C0N])
            del pool_min[:C0N]
            c0s.append(1 - maj)
        else:
            c0_blocks.append(pool_maj[:C0N])
            del pool_maj[:C0N]
            c0s.append(maj)
    rest = pool_min + pool_maj
    for c in range(N_CORES):
        mixed_blocks.append(rest[:MIXED])
        del rest[:MIXED]
    assert not rest
    perms = [np.array(mixed_blocks[c] + c0_blocks[c] + c1_blocks[c],
                      dtype=np.int64) for c in range(N_CORES)]
    for c in range(N_CORES):
        assert perms[c].shape == (BS,)
    return perms, c0s, [maj] * N_CORES


def _host_pack_shared(adjacency, W1, b1, W2, b2, W3, b3, discrete_mask):
    """Pure-layout packing of replicated tables (gather/pad + fp16 round)."""
    f16 = np.float16
    W1t = np.ascontiguousarray(W1.transpose(1, 0, 2))   # [d, n, h]

    w1pk = np.zeros((128, 130 * H), f16)
    w1pk[:, 0:129 * H] = W1t[0:128].reshape(128, 129 * H).astype(f16)

    atpk = np.zeros((128, 130), f16)
    atpk[:, 0:129] = adjacency.T[0:128].astype(f16)

    # node index at (pair j, lhsT column c) / (psum partition c)
    nj = np.zeros((65, 128), np.int64)
    for j in range(65):
        nj[j, 0:64] = min(2 * j, 128)
        nj[j, 64:128] = min(2 * j + 1, 128)
    hh = np.concatenate([np.arange(64), np.arange(64)])
    valid = np.ones((65, 128), bool)
    valid[64, 64:] = False          # pair 64 = node 128 only
    nj_safe = np.where(valid, nj, 0)

    w1lT = (W1t[128][nj_safe, hh[None, :]] * valid).astype(f16)  # [65,128]
    at128T = (adjacency[nj_safe, 128] * valid).astype(f16)
    w1l = np.ascontiguousarray(w1lT.T.astype(np.float32))        # [128,65]
    at128 = np.ascontiguousarray(at128T.T.astype(np.float32))
    b1pack = np.ascontiguousarray(
        (b1[nj_safe, hh[None, :]] * valid).T.astype(np.float32))

    w2pk = np.zeros((128, 65 * H), f16)
    for j in range(64):
        w2pk[0:64, j * H:(j + 1) * H] = W2[2 * j].astype(f16)
        w2pk[64:128, j * H:(j + 1) * H] = W2[2 * j + 1].astype(f16)
    w2pk[0:64, 64 * H:65 * H] = W2[128].astype(f16)

    # h2 partition order: even pair (2j, 2j+1); odd pair swapped (2j+1, 2j)
    def h2order(j):
        if j == 64:
            return [128, -1]
        return [2 * j, 2 * j + 1] if j % 2 == 0 else [2 * j + 1, 2 * j]

    b2pack = np.zeros((128, 65), np.float32)
    w3pk = np.zeros((128, 65 * 128), f16)
    for j in range(65):
        lo, hi = h2order(j)
        b2pack[0:64, j] = b2[lo]
        if hi >= 0:
            b2pack[64:128, j] = b2[hi]
        if j < 64:
            w3pk[0:64, j * 128 + lo] = W3[lo].astype(f16)
            w3pk[64:128, j * 128 + hi] = W3[hi].astype(f16)
        else:
            w3pk[0:64, j * 128] = W3[128].astype(f16)

    return {
        "mt": np.ascontiguousarray(
            discrete_mask[0:128].astype(np.float32).reshape(128, 1)),
        "W1pk": w1pk, "atpk": atpk,
        "w1lT_raw": w1lT, "at128T": at128T,
        "w1l_raw": w1l, "at128": at128, "b1pack": b1pack,
        "W2pk": w2pk, "b2pack": b2pack, "W3pk": w3pk,
        "b3col": np.ascontiguousarray(
            b3[0:128].astype(np.float32).reshape(128, 1)),
        "b3hi": np.array([[b3[128]]], np.float32),
    }


def make_in_maps(inputs, adjacency, W1, b1, W2, b2, W3, b3, discrete_mask):
    """Returns (in_maps, perms): per-core input dicts + row permutations."""
    inputs = np.asarray(inputs, np.float32)
    # the batch-permutation scheme needs column 128 to be a discrete
    # (straight-through binarized) column, which the reference guarantees
    assert int(np.asarray(discrete_mask)[128]) == 1, \
        "kernel requires discrete_mask[128] == 1"
    shared = _host_pack_shared(
        np.asarray(adjacency, np.float32), np.asarray(W1, np.float32),
        np.asarray(b1, np.float32), np.asarray(W2, np.float32),
        np.asarray(b2, np.float32), np.asarray(W3, np.float32),
        np.asarray(b3, np.float32), np.asarray(discrete_mask))
    x128bit = inputs[:, 128] > 0
    perms, c0s, c1s = _plan_shards(x128bit)
    in_maps = []
    for c in range(N_CORES):
        xs = inputs[perms[c]]                       # [1024, 129]
        xt = np.ascontiguousarray(xs.T[0:128].astype(np.float16))
        x128m = np.ascontiguousarray(
            xs[0:MIXED, 128].reshape(1, MIXED).astype(np.float16))
        in_maps.append({
            "xt_raw": xt, "x128m_raw": x128m,
            "c0rep": np.full((128, 1), c0s[c], np.float32),
            "c1rep": np.full((128, 1), c1s[c], np.float32),
            **shared})
    return in_maps, perms


def kernel(inputs, adjacency, W1, b1, W2, b2, W3, b3, discrete_mask,
           trace=False, **trace_kwargs):
    nc = get_nc()
    in_maps, perms = make_in_maps(inputs, adjacency, W1, b1, W2, b2, W3,
                                  b3, discrete_mask)
    res = run_bass_kernel_spmd(nc, in_maps, list(range(N_CORES)),
                               trace=trace, **trace_kwargs)
    out = np.empty((B, D), np.float32)
    for c in range(N_CORES):
        out[perms[c]] = np.ascontiguousarray(res.results[c]["outT"].T)
    if trace:
        kernel.last_results = res
    return out
